# revision 14
# baseline (speedup 1.0000x reference)
"""Grouped-index Conv1D (moe_routing) on 8 TRN2 NeuronCores.

Math:  out[b,d,t] = sum_c sum_k x[b,c,t+k] * W[gi[b,c],d,k] + count0[b]*bias[d]

Device algorithm (per core, 2 batches, data-parallel over batch), bf16:
  1. host precomputes one-hot M[c, 16*(2b+h)+g] = (gi[b,128h+c]==g) in bf16,
     stacked conv weights ws[16k+g, d] = W[g,d,k] in bf16, and
     bc[d, b] = count0[b]*bias[d] in f32.  x is host-cast to bf16 (tolerance
     2e-2 >> bf16 rounding), halving HBM read traffic.
  2. ~8 warmup matmuls on a memset scratch tile run during the fixed ~7 us
     framework preamble + x-load time so the PE HAM clock gate is at 2.4 GHz
     (not the cold 1.2) when real matmuls start.
  3. S[g,t] = sum_c M[c,g]*x[c,t] (PE one-hot matmul, contraction 2x128),
     per-batch tile s[16, 2048], PSUM evacuated in [16,512] chunks on
     alternating DVE/ACT.
  4. swin[16k+g, t] = S[g, t+k]: 7 shifted SBUF->SBUF DMAs per batch (k=0..6
     full width), split 4 on the Sync HWDGE queue + 3 on the GpSimd SWDGE
     queue (DMA-issue instructions cost a flat ~600 ns on their queue; the
     two queues run in parallel and are otherwise idle at that point).
  5. out[d,t] = ws^T @ swin[:, t:t+L] (single matmul, contraction 112,
     N<=512); bias add fused into the PSUM->SBUF evacuation (ACT/DVE
     alternating); output stored bf16, one DMA per batch on GpSimd.

Stage-major emission (S b0, S b1, shifts, conv b0, conv b1) keeps the PE
queue dense: batch 1's S matmuls fill the batch-0 shift-DMA latency.
"""

import sys
import numpy as np

sys.path.insert(0, "/opt/trn_rl_repo")

import ml_dtypes

BS, CH, T = 16, 256, 2048
G, D, K = 16, 64, 7
T_OUT = T - K + 1  # 2042
N_CORES = 8
BPC = BS // N_CORES  # batches per core = 2

BF16 = ml_dtypes.bfloat16

CONV_CHUNKS = [(0, 512), (512, 1024), (1024, 1536), (1536, 2042)]
N_WARMUP = 8

MM_DTYPE = "bf16"

_COMPILED = {}


def _build(cfg: str):
    from concourse import bacc, tile
    import concourse.mybir as mybir

    f32 = mybir.dt.float32
    bf16 = mybir.dt.bfloat16
    add = mybir.AluOpType.add
    act_id = mybir.ActivationFunctionType.Identity
    act_copy = mybir.ActivationFunctionType.Copy

    nc = bacc.Bacc("TRN2", target_bir_lowering=False, debug=False,
                   num_devices=N_CORES)
    # x layout: [b, q, p, h*1024+t']  where channel = 128*h + p and
    # global col = 1024*q + t'  (one 512 KB DMA per (b, q))
    x_ext = nc.dram_tensor("x", [BPC, 2, 128, 2048], bf16,
                           kind="ExternalInput").ap()
    # cs: cols 0-447 hold the 7x-replicated one-hot M7 per (b,h) slice
    # (col 112*(2b+h) + 16k + g = (gi==g)), cols 448-511 ws (rows 0-111).
    # The replicated lhsT makes the S matmul write S to PSUM partitions
    # 0-111 directly (same cycle cost), so the shift DMAs each read their
    # own partition group / SBUF ports instead of all hammering the 2
    # ports that serve partitions 0-15.
    cs_ext = nc.dram_tensor("cs", [128, 576], bf16, kind="ExternalInput").ap()
    bc_ext = nc.dram_tensor("bc", [D, BPC], f32, kind="ExternalInput").ap()
    out_ext = nc.dram_tensor("out", [BPC, D, T_OUT], bf16,
                             kind="ExternalOutput").ap()

    kacc = cfg == "kacc"
    with tile.TileContext(nc) as tc:
        with (
            tc.tile_pool(name="const", bufs=1) as cpool,
            tc.tile_pool(name="work", bufs=2) as wpool,
            tc.tile_pool(name="ps_pool", bufs=4, space="PSUM") as ppool,
            tc.tile_pool(name="po_pool", bufs=4, space="PSUM") as opool,
        ):
            # --- PE warmup: memset scratch, then dummy matmuls to trip the
            # HAM clock gate to 8/8 before the real matmuls arrive ---
            scr = cpool.tile([128, 512], bf16, name="scr")
            nc.gpsimd.memset(scr[:], 0.0)
            for w in range(N_WARMUP):
                pw = opool.tile([G, 512], f32, name=f"pw{w}", tag="po")
                nc.tensor.matmul(pw[:], scr[:, 0:G], scr[:],
                                 start=True, stop=True)

            cs_sb = cpool.tile([128, 576], bf16, name="cs_sb")
            nc.scalar.dma_start(cs_sb[:], cs_ext[:])
            bc_sb = cpool.tile([D, BPC], f32, name="bc_sb")
            nc.scalar.dma_start(bc_sb[:], bc_ext[:])

            # --- x loads: 4 transfers of 512 KB on the Sync queue ---
            xts = [[None, None] for _ in range(BPC)]
            for b in range(BPC):
                for q in range(2):
                    t_ = wpool.tile([128, 2048], bf16, name=f"xt{b}{q}",
                                    tag="xt", bufs=4)
                    nc.sync.dma_start(t_[:], x_ext[b, q])
                    xts[b][q] = t_

            # --- S stage (stage-major: both batches before shifts);
            # output lands 7x-replicated on partitions 0-111 ---
            s_all = []
            for b in range(BPC):
                s_sb = wpool.tile([K * G, T], bf16, name=f"s{b}", tag="s")
                for q in range(2):
                    for cc in range(2):
                        ps = ppool.tile([K * G, 512], f32, name=f"ps{b}{q}{cc}",
                                        tag=f"ps{b}", bufs=2)
                        for h in range(2):
                            m0 = 112 * (2 * b + h)
                            nc.tensor.matmul(
                                ps[:],
                                cs_sb[:, m0:m0 + 112],
                                xts[b][q][:, 1024 * h + 512 * cc:
                                          1024 * h + 512 * cc + 512],
                                start=(h == 0), stop=(h == 1))
                        c0 = 1024 * q + 512 * cc
                        dst = s_sb[:, c0:c0 + 512]
                        if (q + cc) % 2 == 0:
                            nc.vector.tensor_copy(dst, ps[:])
                        else:
                            nc.scalar.activation(dst, ps[:], act_copy)
                s_all.append(s_sb)

            # --- shift replication: per-k SBUF->SBUF DMAs, each group
            # reading its own partitions/ports.  kacc needs only the odd
            # k's (1,3,5) shifted into a 48-row tile; even k's are read
            # straight from s7 by 32-aligned row-tiled conv matmuls ---
            swin_all = []
            if kacc:
                for b in range(BPC):
                    sw3 = wpool.tile([3 * G, T_OUT], bf16, name=f"sw3{b}",
                                     tag="swin")
                    engs3 = [nc.sync, nc.gpsimd, nc.scalar]
                    for j, k in enumerate((1, 3, 5)):
                        engs3[j].dma_start(sw3[G * j:G * (j + 1), :],
                                           s_all[b][G * k:G * (k + 1),
                                                    k:k + T_OUT])
                    swin_all.append(sw3)
            else:
                for b in range(BPC):
                    swin = wpool.tile([K * G, T_OUT], bf16, name=f"swin{b}",
                                      tag="swin")
                    engs = [nc.sync, nc.gpsimd, nc.scalar, nc.gpsimd,
                            nc.sync, nc.sync, nc.scalar]
                    for k in range(K):
                        engs[k].dma_start(swin[G * k:G * (k + 1), :],
                                          s_all[b][G * k:G * (k + 1),
                                                   k:k + T_OUT])
                    swin_all.append(swin)
            # spacer matmuls keep the HAM clock gate warm across the
            # S->conv dependency gap
            for w in range(3):
                pw = opool.tile([G, 512], f32, name=f"sp{w}", tag="po")
                nc.tensor.matmul(pw[:], scr[:, 0:G], scr[:],
                                 start=True, stop=True)

            # --- conv + bias + store ---
            for b in range(BPC):
                osb = wpool.tile([D, T_OUT], bf16, name=f"osb{b}", tag="osb")
                for ci, (c0, c1) in enumerate(CONV_CHUNKS):
                    L = c1 - c0
                    po = opool.tile([D, 512], f32, name=f"po{b}{ci}", tag="po")
                    if kacc:
                        # even k straight from the replicated s7 tile:
                        # 4 row-strips run concurrently in the PE array
                        for k in (0, 2, 4, 6):
                            nc.tensor.matmul(
                                po[:, :L],
                                cs_sb[G * k:G * (k + 1), 448:512],
                                s_all[b][G * k:G * (k + 1), c0 + k:c1 + k],
                                start=(k == 0), stop=False,
                                tile_position=(16 * k, 0))
                        nc.tensor.matmul(po[:, :L],
                                         cs_sb[0:3 * G, 512:576],
                                         swin_all[b][:, c0:c1],
                                         start=False, stop=True)
                    else:
                        nc.tensor.matmul(po[:, :L], cs_sb[0:K * G, 448:512],
                                         swin_all[b][:, c0:c1],
                                         start=True, stop=True)
                    if ci % 2 == 0:
                        nc.vector.tensor_scalar(out=osb[:, c0:c1],
                                                in0=po[:, :L],
                                                scalar1=bc_sb[:, b:b + 1],
                                                scalar2=None, op0=add)
                    else:
                        nc.scalar.activation(osb[:, c0:c1], po[:, :L], act_id,
                                             bias=bc_sb[:, b:b + 1])
                nc.gpsimd.dma_start(out_ext[b, :, 0:1024], osb[:, 0:1024])
                nc.sync.dma_start(out_ext[b, :, 1024:T_OUT],
                                  osb[:, 1024:T_OUT])

    nc.compile()
    return nc


def _get_nc(mm_dtype: str):
    if mm_dtype not in _COMPILED:
        _COMPILED[mm_dtype] = _build(mm_dtype)
    return _COMPILED[mm_dtype]


def _run(x, group_idxs, W, bias, mm_dtype=None, trace=False, tmpdir=None):
    from concourse.bass_utils import run_bass_kernel_spmd

    x = np.asarray(x, dtype=np.float32)
    gi = np.asarray(group_idxs)
    W = np.asarray(W, dtype=np.float32)
    bias = np.asarray(bias, dtype=np.float32)

    # x per core: [2, 256, 2048] -> [b, h, p, q, t'] -> [b, q, p, h, t']
    xr = x.reshape(BS // BPC, BPC, 2, 128, 2, 1024).transpose(0, 1, 4, 3, 2, 5)
    xr = np.ascontiguousarray(xr.reshape(BS // BPC, BPC, 2, 128, 2048)
                              ).astype(BF16)
    # one-hot M7: [bs, ch] -> per core [128, 4*112], col 112*(2b+h)+16k+g
    oh = (gi[..., None] == np.arange(G)).astype(np.float32)  # [bs, 256, 16]
    ohm = oh.reshape(BS // BPC, BPC, 2, 128, G).transpose(0, 3, 1, 2, 4)
    ohm = np.tile(ohm, (1, 1, 1, 1, K))  # [cores,128,2,2,112]
    ohm = ohm.reshape(BS // BPC, 128, 4 * K * G)
    # ws[k*16+g, d] = W[g, d, k], zero-padded to 128 rows
    wsk = W.transpose(2, 0, 1).reshape(K, G, D)  # [k, g, d]
    ws = np.zeros((128, D), dtype=np.float32)
    ws[:K * G] = wsk.reshape(K * G, D)
    wso = np.zeros((128, D), dtype=np.float32)  # odd k at rows 16j+g
    wso[:3 * G] = wsk[1::2].reshape(3 * G, D)
    cs = np.concatenate([ohm, np.broadcast_to(ws, (BS // BPC, 128, D)),
                         np.broadcast_to(wso, (BS // BPC, 128, D))],
                        axis=2).astype(BF16)  # [cores, 128, 576]
    # bc[d, b] = count0[b] * bias[d]
    count0 = (gi == 0).sum(axis=1).astype(np.float32)  # [bs]
    bc = (count0[None, :] * bias[:, None]).astype(np.float32)  # [64, bs]
    bc = bc.reshape(D, BS // BPC, BPC).transpose(1, 0, 2)  # [cores, 64, 2]

    nc = _get_nc(mm_dtype or MM_DTYPE)
    in_maps = []
    for i in range(N_CORES):
        in_maps.append({
            "x": xr[i],
            "cs": np.ascontiguousarray(cs[i]),
            "bc": np.ascontiguousarray(bc[i]),
        })
    res = run_bass_kernel_spmd(nc, in_maps, core_ids=list(range(N_CORES)),
                               trace=trace, tmpdir=tmpdir)
    out = np.concatenate([np.asarray(r["out"], dtype=np.float32)
                          for r in res.results], axis=0)
    assert out.shape == (BS, D, T_OUT)
    return out, res


def kernel(x, group_idxs, W, bias):
    out, _ = _run(x, group_idxs, W, bias)
    return out


# revision 15
# speedup vs baseline: 1.0282x; 1.0282x over previous
"""Grouped-index Conv1D (moe_routing) on 8 TRN2 NeuronCores.

Math:  out[b,d,t] = sum_c sum_k x[b,c,t+k] * W[gi[b,c],d,k] + count0[b]*bias[d]

Device algorithm (per core, 2 batches, data-parallel over batch), bf16:
  1. host precomputes one-hot M[c, 16*(2b+h)+g] = (gi[b,128h+c]==g) in bf16,
     stacked conv weights ws[16k+g, d] = W[g,d,k] in bf16, and
     bc[d, b] = count0[b]*bias[d] in f32.  x is host-cast to bf16 (tolerance
     2e-2 >> bf16 rounding), halving HBM read traffic.
  2. ~8 warmup matmuls on a memset scratch tile run during the fixed ~7 us
     framework preamble + x-load time so the PE HAM clock gate is at 2.4 GHz
     (not the cold 1.2) when real matmuls start.
  3. S[g,t] = sum_c M[c,g]*x[c,t] (PE one-hot matmul, contraction 2x128),
     per-batch tile s[16, 2048], PSUM evacuated in [16,512] chunks on
     alternating DVE/ACT.
  4. swin[16k+g, t] = S[g, t+k]: 7 shifted SBUF->SBUF DMAs per batch (k=0..6
     full width), split 4 on the Sync HWDGE queue + 3 on the GpSimd SWDGE
     queue (DMA-issue instructions cost a flat ~600 ns on their queue; the
     two queues run in parallel and are otherwise idle at that point).
  5. out[d,t] = ws^T @ swin[:, t:t+L] (single matmul, contraction 112,
     N<=512); bias add fused into the PSUM->SBUF evacuation (ACT/DVE
     alternating); output stored bf16, one DMA per batch on GpSimd.

Stage-major emission (S b0, S b1, shifts, conv b0, conv b1) keeps the PE
queue dense: batch 1's S matmuls fill the batch-0 shift-DMA latency.
"""

import sys
import numpy as np

sys.path.insert(0, "/opt/trn_rl_repo")

import ml_dtypes

BS, CH, T = 16, 256, 2048
G, D, K = 16, 64, 7
T_OUT = T - K + 1  # 2042
N_CORES = 8
BPC = BS // N_CORES  # batches per core = 2

BF16 = ml_dtypes.bfloat16

CONV_CHUNKS = [(0, 512), (512, 1024), (1024, 1536), (1536, 2042)]
N_WARMUP = 8

MM_DTYPE = "bf16"

_COMPILED = {}


def _build(cfg: str):
    from concourse import bacc, tile
    import concourse.mybir as mybir

    f32 = mybir.dt.float32
    bf16 = mybir.dt.bfloat16
    add = mybir.AluOpType.add
    act_id = mybir.ActivationFunctionType.Identity
    act_copy = mybir.ActivationFunctionType.Copy

    nc = bacc.Bacc("TRN2", target_bir_lowering=False, debug=False,
                   num_devices=N_CORES)
    # x layout: [b, q, p, h*1024+t']  where channel = 128*h + p and
    # global col = 1024*q + t'  (one 512 KB DMA per (b, q))
    x_ext = nc.dram_tensor("x", [BPC, 2, 128, 2048], bf16,
                           kind="ExternalInput").ap()
    # cs: cols 0-447 hold the 7x-replicated one-hot M7 per (b,h) slice
    # (col 112*(2b+h) + 16k + g = (gi==g)), cols 448-511 ws (rows 0-111).
    # The replicated lhsT makes the S matmul write S to PSUM partitions
    # 0-111 directly (same cycle cost), so the shift DMAs each read their
    # own partition group / SBUF ports instead of all hammering the 2
    # ports that serve partitions 0-15.
    cs_ext = nc.dram_tensor("cs", [128, 576], bf16, kind="ExternalInput").ap()
    bc_ext = nc.dram_tensor("bc", [D, BPC], f32, kind="ExternalInput").ap()
    out_ext = nc.dram_tensor("out", [BPC, D, T_OUT], bf16,
                             kind="ExternalOutput").ap()

    kacc = cfg == "kacc"
    with tile.TileContext(nc) as tc:
        with (
            tc.tile_pool(name="const", bufs=1) as cpool,
            tc.tile_pool(name="work", bufs=2) as wpool,
            tc.tile_pool(name="ps_pool", bufs=4, space="PSUM") as ppool,
            tc.tile_pool(name="po_pool", bufs=4, space="PSUM") as opool,
        ):
            # --- PE warmup: memset scratch, then dummy matmuls to trip the
            # HAM clock gate to 8/8 before the real matmuls arrive ---
            scr = cpool.tile([128, 512], bf16, name="scr")
            nc.gpsimd.memset(scr[:], 0.0)
            for w in range(N_WARMUP):
                pw = opool.tile([G, 512], f32, name=f"pw{w}", tag="po")
                nc.tensor.matmul(pw[:], scr[:, 0:G], scr[:],
                                 start=True, stop=True)

            cs_sb = cpool.tile([128, 576], bf16, name="cs_sb")
            nc.scalar.dma_start(cs_sb[:], cs_ext[:])
            bc_sb = cpool.tile([D, BPC], f32, name="bc_sb")
            nc.scalar.dma_start(bc_sb[:], bc_ext[:])

            # --- x loads: 4 transfers of 512 KB on the Sync queue ---
            xts = [[None, None] for _ in range(BPC)]
            for b in range(BPC):
                for q in range(2):
                    t_ = wpool.tile([128, 2048], bf16, name=f"xt{b}{q}",
                                    tag="xt", bufs=4)
                    nc.sync.dma_start(t_[:], x_ext[b, q])
                    xts[b][q] = t_

            # --- S stage (stage-major: both batches before shifts);
            # output lands 7x-replicated on partitions 0-111 ---
            s_all = []
            for b in range(BPC):
                s_sb = wpool.tile([K * G, T], bf16, name=f"s{b}", tag="s")
                for q in range(2):
                    for cc in range(2):
                        ps = ppool.tile([K * G, 512], f32, name=f"ps{b}{q}{cc}",
                                        tag=f"ps{b}", bufs=2)
                        for h in range(2):
                            m0 = 112 * (2 * b + h)
                            nc.tensor.matmul(
                                ps[:],
                                cs_sb[:, m0:m0 + 112],
                                xts[b][q][:, 1024 * h + 512 * cc:
                                          1024 * h + 512 * cc + 512],
                                start=(h == 0), stop=(h == 1))
                        c0 = 1024 * q + 512 * cc
                        dst = s_sb[:, c0:c0 + 512]
                        if (q + cc) % 2 == 0:
                            nc.vector.tensor_copy(dst, ps[:])
                        else:
                            nc.scalar.activation(dst, ps[:], act_copy)
                s_all.append(s_sb)

            # --- shift replication: per-k SBUF->SBUF DMAs, each group
            # reading its own partitions/ports.  kacc needs only the odd
            # k's (1,3,5) shifted into a 48-row tile; even k's are read
            # straight from s7 by 32-aligned row-tiled conv matmuls ---
            swin_all = []
            if kacc:
                for b in range(BPC):
                    sw3 = wpool.tile([3 * G, T_OUT], bf16, name=f"sw3{b}",
                                     tag="swin")
                    engs3 = [nc.sync, nc.gpsimd, nc.scalar]
                    for j, k in enumerate((1, 3, 5)):
                        engs3[j].dma_start(sw3[G * j:G * (j + 1), :],
                                           s_all[b][G * k:G * (k + 1),
                                                    k:k + T_OUT])
                    swin_all.append(sw3)
            else:
                for b in range(BPC):
                    swin = wpool.tile([K * G, T_OUT], bf16, name=f"swin{b}",
                                      tag="swin")
                    engs = [nc.sync, nc.gpsimd, nc.sync, nc.gpsimd,
                            nc.sync, nc.gpsimd, nc.sync]
                    for k in range(K):
                        engs[k].dma_start(swin[G * k:G * (k + 1), :],
                                          s_all[b][G * k:G * (k + 1),
                                                   k:k + T_OUT])
                    swin_all.append(swin)
            # spacer matmuls keep the HAM clock gate warm across the
            # S->conv dependency gap
            for w in range(3):
                pw = opool.tile([G, 512], f32, name=f"sp{w}", tag="po")
                nc.tensor.matmul(pw[:], scr[:, 0:G], scr[:],
                                 start=True, stop=True)

            # --- conv + bias + store ---
            for b in range(BPC):
                osb = wpool.tile([D, T_OUT], bf16, name=f"osb{b}", tag="osb")
                for ci, (c0, c1) in enumerate(CONV_CHUNKS):
                    L = c1 - c0
                    po = opool.tile([D, 512], f32, name=f"po{b}{ci}", tag="po")
                    if kacc:
                        # even k straight from the replicated s7 tile:
                        # 4 row-strips run concurrently in the PE array
                        for k in (0, 2, 4, 6):
                            nc.tensor.matmul(
                                po[:, :L],
                                cs_sb[G * k:G * (k + 1), 448:512],
                                s_all[b][G * k:G * (k + 1), c0 + k:c1 + k],
                                start=(k == 0), stop=False,
                                tile_position=(16 * k, 0))
                        nc.tensor.matmul(po[:, :L],
                                         cs_sb[0:3 * G, 512:576],
                                         swin_all[b][:, c0:c1],
                                         start=False, stop=True)
                    else:
                        nc.tensor.matmul(po[:, :L], cs_sb[0:K * G, 448:512],
                                         swin_all[b][:, c0:c1],
                                         start=True, stop=True)
                    if ci % 2 == 0:
                        nc.vector.tensor_scalar(out=osb[:, c0:c1],
                                                in0=po[:, :L],
                                                scalar1=bc_sb[:, b:b + 1],
                                                scalar2=None, op0=add)
                    else:
                        nc.scalar.activation(osb[:, c0:c1], po[:, :L], act_id,
                                             bias=bc_sb[:, b:b + 1])
                nc.gpsimd.dma_start(out_ext[b, :, 0:1024], osb[:, 0:1024])
                nc.sync.dma_start(out_ext[b, :, 1024:T_OUT],
                                  osb[:, 1024:T_OUT])

    nc.compile()
    return nc


def _get_nc(mm_dtype: str):
    if mm_dtype not in _COMPILED:
        _COMPILED[mm_dtype] = _build(mm_dtype)
    return _COMPILED[mm_dtype]


def _run(x, group_idxs, W, bias, mm_dtype=None, trace=False, tmpdir=None):
    from concourse.bass_utils import run_bass_kernel_spmd

    x = np.asarray(x, dtype=np.float32)
    gi = np.asarray(group_idxs)
    W = np.asarray(W, dtype=np.float32)
    bias = np.asarray(bias, dtype=np.float32)

    # x per core: [2, 256, 2048] -> [b, h, p, q, t'] -> [b, q, p, h, t']
    xr = x.reshape(BS // BPC, BPC, 2, 128, 2, 1024).transpose(0, 1, 4, 3, 2, 5)
    xr = np.ascontiguousarray(xr.reshape(BS // BPC, BPC, 2, 128, 2048)
                              ).astype(BF16)
    # one-hot M7: [bs, ch] -> per core [128, 4*112], col 112*(2b+h)+16k+g
    oh = (gi[..., None] == np.arange(G)).astype(np.float32)  # [bs, 256, 16]
    ohm = oh.reshape(BS // BPC, BPC, 2, 128, G).transpose(0, 3, 1, 2, 4)
    ohm = np.tile(ohm, (1, 1, 1, 1, K))  # [cores,128,2,2,112]
    ohm = ohm.reshape(BS // BPC, 128, 4 * K * G)
    # ws[k*16+g, d] = W[g, d, k], zero-padded to 128 rows
    wsk = W.transpose(2, 0, 1).reshape(K, G, D)  # [k, g, d]
    ws = np.zeros((128, D), dtype=np.float32)
    ws[:K * G] = wsk.reshape(K * G, D)
    wso = np.zeros((128, D), dtype=np.float32)  # odd k at rows 16j+g
    wso[:3 * G] = wsk[1::2].reshape(3 * G, D)
    cs = np.concatenate([ohm, np.broadcast_to(ws, (BS // BPC, 128, D)),
                         np.broadcast_to(wso, (BS // BPC, 128, D))],
                        axis=2).astype(BF16)  # [cores, 128, 576]
    # bc[d, b] = count0[b] * bias[d]
    count0 = (gi == 0).sum(axis=1).astype(np.float32)  # [bs]
    bc = (count0[None, :] * bias[:, None]).astype(np.float32)  # [64, bs]
    bc = bc.reshape(D, BS // BPC, BPC).transpose(1, 0, 2)  # [cores, 64, 2]

    nc = _get_nc(mm_dtype or MM_DTYPE)
    in_maps = []
    for i in range(N_CORES):
        in_maps.append({
            "x": xr[i],
            "cs": np.ascontiguousarray(cs[i]),
            "bc": np.ascontiguousarray(bc[i]),
        })
    res = run_bass_kernel_spmd(nc, in_maps, core_ids=list(range(N_CORES)),
                               trace=trace, tmpdir=tmpdir)
    out = np.concatenate([np.asarray(r["out"], dtype=np.float32)
                          for r in res.results], axis=0)
    assert out.shape == (BS, D, T_OUT)
    return out, res


def kernel(x, group_idxs, W, bias):
    out, _ = _run(x, group_idxs, W, bias)
    return out
